# revision 1
# baseline (speedup 1.0000x reference)
"""CoxLoss (nn_CoxLoss) Trainium2 kernel: 2-level histogram, suffix form, 8-way SPMD.

key_j = (a_j, v_j), a = floor(s*64), v = floor(u), u = s*8192 - 128*a.
risk_i = sum_j w_j [key_j >= key_i]  (13-bit key order; ties/diagonal exact,
remaining error = distinct-value same-key pairs ~ 1e-4 relative on the loss)

  Ct[v, a]   = sum_j w_j [v_j == v][a_j == a]      (PE one-hot matmul, sharded
                                                    over j; AllGather of the
                                                    16KB bf16 partial + on-chip
                                                    tree-sum — fewer fabric
                                                    steps than AllReduce)
  H_A[a]     = sum_v Ct[v, a]                      (PE, ones rhs, written at
                                                    partitions 64..127)
  r1[a, i]   = sum_v Ct[v, a] [v >= v_i]           (PE, suffix mask rhs)
  risk_i     = r1[a_i, i] + sum_{b > a_i} H_A[b]   (one-hot select + H*tail
                                                    mask stacked [64+64, i],
                                                    one reduce-matmul per
                                                    128-col block; both sums
                                                    all-positive => bf16 safe)
  loss_part  = -(1/N) sum_i cen_i (ln w_i - ln risk_i), host sums 8 partials.

Scheduling: input rows are DMA'd unreplicated and broadcast on-chip via K=1
matmuls (DMA-broadcast is ~75GB/s); the i-side mask tiles overlap the
collective's skew-barrier window; everything pre-collective only needs to
beat the ~28us runtime barrier, so the critical path is the post-collective
chain (gather-load, tree-sum, 5 matmuls sharing one stationary, stacked
mask-products, 16 reduce matmuls, ln epilogue).

All quantities summed with positive terms only (no telescoping differences),
so single bf16 matmuls suffice end-to-end (validated ~1e-4 rel err on HW,
tolerance 2e-2).
"""
import numpy as np
import concourse.bass as bass
import concourse.mybir as mybir
from concourse.tile import TileContext
from concourse.bass_utils import run_bass_kernel_spmd

F32 = mybir.dt.float32
I32 = mybir.dt.int32
BF16 = mybir.dt.bfloat16
AF = mybir.ActivationFunctionType
ALU = mybir.AluOpType

N = 16384
P = 128
NCORES = 8
MY = N // NCORES          # 2048 rows per core
JCH = MY // P             # 16 j-chunks per core
B1 = 64                   # level-1 bins (a)
B2 = 128                  # level-2 bins (v)
SCALE2 = float(B1 * B2)   # 8192
QF = 512
NQ = MY // QF             # 4 i-tiles

# in_pack columns
C_S = 0
C_TH = JCH
C_CEN = 2 * JCH
C_VP1 = 3 * JCH           # p (dct_c is_le threshold grid)
C_IBC = C_VP1 + 1         # (p-64)/64 (dab_c threshold)
C_BG = C_IBC + 1          # p if p<64 else -5 (onehA is_equal grid)
C_ONE = C_BG + 1          # 1.0
C_SCL = C_ONE + 1         # 1/N (final reduction scale)
PACKW = C_SCL + 1


def legalize_waits(nc, max_waits=1):
    """Insert same-engine Drains carrying excess sync waits immediately
    before each offending instruction (walrus accepts ~1 wait/instr here)."""
    fn = nc.m.functions[0]
    for blk in fn.blocks:
        insts = blk.instructions
        out_list = []
        changed = False
        for ins in insts:
            si = ins.sync_info
            if si is not None and len(si.on_wait) > max_waits:
                waits = list(si.on_wait)
                keep = waits[:max_waits]
                for k, w in enumerate(waits[max_waits:]):
                    d = mybir.InstDrain(name=f"{ins.name}-w{k}", ins=[], outs=[])
                    d.engine = ins.engine
                    d.sync_info = mybir.SyncInfo(on_wait=[w], on_update=[])
                    out_list.append(d)
                si.on_wait = keep
                ins.sync_info = si
                changed = True
            out_list.append(ins)
        if changed:
            blk.instructions = out_list


DEBUG = False
# Lean floor: floor(x) = cvt_rne(x - 0.5) — matches HW convert semantics.
# Robust floor: cvt + compare + fixup — also correct under the simulator's
# truncating convert. Numerically identical on HW except exact-integer ties
# (where robust is exact); ~4 extra DVE ops, all hidden in the AR window.
ROBUST_FLOOR = False
# AllGather (3 RDH steps) + on-chip tree-sum instead of AllReduce (6 steps):
# the collective is latency-bound, so fewer fabric steps win.
USE_AG = True


def build(no_ar=False):
    nc = bass.Bass()
    in_pack = nc.dram_tensor("in_pack", [P, PACKW], F32, kind="ExternalInput")
    in_row = nc.dram_tensor("in_row", [1, MY], F32, kind="ExternalInput")
    in_iota = nc.dram_tensor("in_iota", [1, B2 + B1], F32, kind="ExternalInput")
    out = nc.dram_tensor("partial", [1, 1], F32, kind="ExternalOutput")
    if DEBUG:
        dbg = nc.dram_tensor("dbg", [P, JCH + 1 + 2 * B1], F32,
                             kind="ExternalOutput")

    ct_dram = nc.dram_tensor("ct_dram", [B2, B1], BF16)
    ct_sh = nc.dram_tensor("ct_sh", [B2, B1], BF16, addr_space="Shared")
    ct_ag = nc.dram_tensor("ct_ag", [NCORES, B2, B1], BF16, addr_space="Shared")

    with TileContext(nc) as tc:
        with (
            tc.tile_pool(name="const", bufs=1) as cpool,
            tc.tile_pool(name="jstair", bufs=16) as jpool,
            tc.tile_pool(name="small", bufs=1) as spool,
            tc.tile_pool(name="stack", bufs=2) as stpool,
            tc.tile_pool(name="pct", bufs=1, space="PSUM") as pct,
            tc.tile_pool(name="pha", bufs=1, space="PSUM") as pha,
            tc.tile_pool(name="pr1", bufs=2, space="PSUM") as pr1,
            tc.tile_pool(name="prk", bufs=1, space="PSUM") as prk,
            tc.tile_pool(name="pfin", bufs=1, space="PSUM") as pfin,
            tc.tile_pool(name="pbc", bufs=2, space="PSUM") as pbc,
        ):
            # ---------------- input DMAs (row-shaped only; broadcasts are
            # done on-chip via K=1 matmuls — the DMA-broadcast path is slow)
            pack = cpool.tile([P, PACKW], F32)
            nc.gpsimd.dma_start(out=pack, in_=in_pack[:, :])
            iota_row = cpool.tile([1, B2 + B1], F32)
            nc.sync.dma_start(out=iota_row, in_=in_iota[:, :])
            s_row = cpool.tile([1, MY], F32)
            nc.sync.dma_start(out=s_row, in_=in_row[:, :])

            s_cols = pack[:, C_S:C_S + JCH]
            th_cols = pack[:, C_TH:C_TH + JCH]
            cen_cols = pack[:, C_CEN:C_CEN + JCH]
            vp1_col = pack[:, C_VP1:C_VP1 + 1]
            ibc_col = pack[:, C_IBC:C_IBC + 1]
            bg_col = pack[:, C_BG:C_BG + 1]
            ones_col = pack[:, C_ONE:C_ONE + 1]
            scl_col = pack[:, C_SCL:C_SCL + 1]

            # sigmoid first on ACT (gates the j one-hots)
            w_col = cpool.tile([P, JCH], F32)
            nc.scalar.activation(out=w_col, in_=th_cols, func=AF.Sigmoid)

            ones_row = cpool.tile([1, P], F32)
            nc.vector.memset(ones_row, 1.0)

            # on-chip broadcast: out[p, c] = row[0, c]
            iota = cpool.tile([P, B2 + B1], F32)
            ib_ps = pbc.tile([P, B2 + B1], F32, tag="bc", name="ibc")
            nc.tensor.matmul(ib_ps[:, :], ones_row[:1, :], iota_row[:1, :],
                             start=True, stop=True)
            nc.scalar.copy(iota, ib_ps[:, :])
            s_rep = cpool.tile([P, MY], F32)
            for h in range(NQ):
                sb_ps = pbc.tile([P, QF], F32, tag="bc", name=f"sbc{h}")
                nc.tensor.matmul(sb_ps[:, :], ones_row[:1, :],
                                 s_row[:1, h * QF:(h + 1) * QF],
                                 start=True, stop=True)
                nc.scalar.copy(s_rep[:, h * QF:(h + 1) * QF], sb_ps[:, :])

            iotaT = iota[:, 0:B2]
            iotaB = iota[:, B2:B2 + B1]

            ones_bf = cpool.tile([P, 1], BF16)
            nc.vector.tensor_copy(ones_bf, ones_col)

            # floor(in*scale) for nonneg values, f32 result.
            def emit_floor(in_ap, scale, width, nm):
                if ROBUST_FLOOR:
                    x = cpool.tile([P, width], F32, name=f"{nm}x")
                    nc.vector.tensor_scalar(out=x, in0=in_ap, scalar1=scale,
                                            scalar2=None, op0=ALU.mult)
                    yi = cpool.tile([P, width], I32, name=f"{nm}i")
                    nc.vector.tensor_copy(yi, x)
                    f0 = cpool.tile([P, width], F32, name=f"{nm}f0")
                    nc.vector.tensor_copy(f0, yi)
                    m = cpool.tile([P, width], F32, name=f"{nm}m")
                    nc.vector.tensor_tensor(out=m, in0=f0, in1=x, op=ALU.is_gt)
                    f = cpool.tile([P, width], F32, name=f"{nm}f")
                    nc.vector.tensor_tensor(out=f, in0=f0, in1=m,
                                            op=ALU.subtract)
                    return f
                w = cpool.tile([P, width], F32, name=f"{nm}w")
                nc.vector.tensor_scalar(out=w, in0=in_ap, scalar1=scale,
                                        scalar2=0.5, op0=ALU.mult,
                                        op1=ALU.subtract)
                yi = cpool.tile([P, width], I32, name=f"{nm}i")
                nc.vector.tensor_copy(yi, w)
                f = cpool.tile([P, width], F32, name=f"{nm}f")
                nc.vector.tensor_copy(f, yi)
                return f

            # ---------------- j prologue on [128, 16] (floor chains)
            af = emit_floor(s_cols, float(B1), JCH, "a")
            t3c = cpool.tile([P, JCH], F32)
            nc.vector.tensor_scalar(out=t3c, in0=s_cols, scalar1=SCALE2,
                                    scalar2=None, op0=ALU.mult)
            u_col = cpool.tile([P, JCH], F32)
            nc.vector.scalar_tensor_tensor(out=u_col, in0=af,
                                           scalar=-float(B2), in1=t3c,
                                           op0=ALU.mult, op1=ALU.add)
            vf = emit_floor(u_col, 1.0, JCH, "v")

            # ---------------- j one-hots (ohu on DVE, ohaw on gpsimd) + Ct
            ct_ps = pct.tile([P, B1], F32, tag="ct")
            for jc in range(JCH):
                ohu = jpool.tile([P, B2], BF16, tag="ohu", name=f"ohu{jc}")
                nc.vector.tensor_scalar(out=ohu, in0=iotaT,
                                        scalar1=vf[:, jc:jc + 1],
                                        scalar2=None, op0=ALU.is_equal)
                ohaw = jpool.tile([P, B1], BF16, tag="ohaw", name=f"ohaw{jc}")
                nc.vector.tensor_scalar(out=ohaw, in0=iotaB,
                                        scalar1=af[:, jc:jc + 1],
                                        scalar2=w_col[:, jc:jc + 1],
                                        op0=ALU.is_equal, op1=ALU.mult)
                nc.tensor.matmul(ct_ps[:, :], ohu, ohaw,
                                 start=(jc == 0), stop=(jc == JCH - 1))

            # Ct -> SBUF(bf16) -> DRAM -> collective -> back
            ct_sb = spool.tile([P, B1], BF16)
            nc.scalar.copy(ct_sb, ct_ps[:, :])
            nc.gpsimd.dma_start(out=ct_dram[:, :], in_=ct_sb)
            if no_ar:
                nc.gpsimd.dma_start(out=ct_sh[:, :], in_=ct_dram[:, :])
            elif USE_AG:
                nc.gpsimd.collective_compute(
                    "AllGather", ALU.bypass,
                    ins=[ct_dram[:, :]], outs=[ct_ag[:, :, :]],
                    replica_groups=[list(range(NCORES))])
            else:
                nc.gpsimd.collective_compute(
                    "AllReduce", ALU.add,
                    ins=[ct_dram[:, :]], outs=[ct_sh[:, :]],
                    replica_groups=[list(range(NCORES))])

            # ---------------- i-side tiles (overlap with AR)
            af2 = emit_floor(s_rep, float(B1), MY, "ai")
            ut32 = cpool.tile([P, MY], F32)
            nc.scalar.activation(out=ut32, in_=s_rep, func=AF.Copy,
                                 scale=SCALE2)
            u_rep = cpool.tile([P, MY], F32)
            nc.vector.scalar_tensor_tensor(out=u_rep, in0=af2,
                                           scalar=-float(B2), in1=ut32,
                                           op0=ALU.mult, op1=ALU.add)
            # v_i via the identical floor chain as the j side, then integer
            # compare [v_i <= p]  (consistent even on RNE ties)
            vf2 = emit_floor(u_rep, 1.0, MY, "vi")
            dct_c = cpool.tile([P, MY], BF16)
            nc.vector.tensor_scalar(out=dct_c, in0=vf2, scalar1=vp1_col,
                                    scalar2=None, op0=ALU.is_le)
            onehA = cpool.tile([P, MY], BF16)
            nc.vector.tensor_scalar(out=onehA, in0=af2, scalar1=bg_col,
                                    scalar2=None, op0=ALU.is_equal)
            dab_c = cpool.tile([P, MY], BF16)
            nc.vector.tensor_scalar(out=dab_c, in0=s_rep, scalar1=ibc_col,
                                    scalar2=None, op0=ALU.is_lt)

            # ---------------- post-AR
            ctb = spool.tile([P, B1], BF16)
            if no_ar:
                nc.gpsimd.dma_start(out=ctb, in_=ct_sh[:, :])
            elif USE_AG:
                # load the 8 gathered partials (two transposed-AP DMAs on
                # separate queues), tree-sum on DVE
                ct8 = spool.tile([P, NCORES * B1], BF16)
                HN = NCORES // 2
                nc.gpsimd.dma_start(
                    out=ct8[:, 0:HN * B1],
                    in_=ct_ag[0:HN, :, :].transpose([1, 0, 2]))
                nc.sync.dma_start(
                    out=ct8[:, HN * B1:NCORES * B1],
                    in_=ct_ag[HN:NCORES, :, :].transpose([1, 0, 2]))
                h4 = spool.tile([P, 4 * B1], F32)
                nc.vector.tensor_tensor(out=h4, in0=ct8[:, 0:4 * B1],
                                        in1=ct8[:, 4 * B1:8 * B1], op=ALU.add)
                h2 = spool.tile([P, 2 * B1], F32)
                nc.vector.tensor_tensor(out=h2, in0=h4[:, 0:2 * B1],
                                        in1=h4[:, 2 * B1:4 * B1], op=ALU.add)
                nc.vector.tensor_tensor(out=ctb, in0=h2[:, 0:B1],
                                        in1=h2[:, B1:2 * B1], op=ALU.add)
            else:
                nc.gpsimd.dma_start(out=ctb, in_=ct_sh[:, :])

            # H_A column written directly at partitions 64..127 (matmul out
            # AP partition offset) so hprod's per-partition scalar aligns;
            # hprod reads it straight from PSUM
            ha_ps = pha.tile([P, 1], F32, tag="ha")
            nc.tensor.matmul(ha_ps[B1:P, :], ctb, ones_bf,
                             start=True, stop=True)
            ha2 = ha_ps

            risk_pm = prk.tile([P, JCH], F32, tag="risk")
            for it in range(NQ):
                r1 = pr1.tile([B1, QF], F32, tag="r1", name=f"r1_{it}")
                nc.tensor.matmul(r1[:, :], ctb,
                                 dct_c[:, it * QF:(it + 1) * QF],
                                 start=True, stop=True)
                stk = stpool.tile([P, QF], BF16, tag="stk", name=f"stk{it}")
                nc.vector.tensor_scalar(
                    out=stk[B1:P, :], in0=dab_c[B1:P, it * QF:(it + 1) * QF],
                    scalar1=ha2[B1:P, :], scalar2=None, op0=ALU.mult)
                nc.vector.tensor_tensor(
                    out=stk[0:B1, :], in0=r1[:, :],
                    in1=onehA[0:B1, it * QF:(it + 1) * QF], op=ALU.mult)
                for k in range(QF // P):
                    col = it * (QF // P) + k
                    nc.tensor.matmul(risk_pm[:, col:col + 1],
                                     stk[:, k * P:(k + 1) * P], ones_bf,
                                     start=True, stop=True,
                                     skip_group_check=True)

            # ---------------- epilogue on [128, 16]
            lnr = spool.tile([P, JCH], F32)
            nc.scalar.activation(out=lnr, in_=risk_pm[:, :], func=AF.Ln)
            lnw = spool.tile([P, JCH], F32)
            nc.scalar.activation(out=lnw, in_=w_col, func=AF.Ln)
            dd = spool.tile([P, JCH], F32)
            nc.vector.scalar_tensor_tensor(out=dd, in0=lnw, scalar=-1.0,
                                           in1=lnr, op0=ALU.mult, op1=ALU.add)
            tt = spool.tile([P, JCH], F32)
            nc.vector.tensor_tensor(out=tt, in0=dd, in1=cen_cols, op=ALU.mult)
            red = spool.tile([P, 1], F32)
            nc.vector.tensor_reduce(out=red, in_=tt, op=ALU.add,
                                    axis=mybir.AxisListType.X)
            fin = pfin.tile([1, 1], F32, tag="fin")
            nc.tensor.matmul(fin[:1, :], red, scl_col, start=True, stop=True)
            part = spool.tile([1, 1], F32)
            nc.vector.tensor_copy(part[:1, :], fin[:1, :])
            nc.gpsimd.dma_start(out=out[:, :], in_=part[:1, :])

            if DEBUG:
                dbg_sb = spool.tile([P, JCH + 1 + 2 * B1], F32)
                nc.vector.tensor_copy(dbg_sb[:, 0:JCH], risk_pm[:, :])
                nc.vector.memset(dbg_sb[:, JCH:JCH + 1], 0.0)
                nc.vector.tensor_copy(dbg_sb[B1:P, JCH:JCH + 1], ha2[B1:P, :])
                nc.vector.tensor_copy(dbg_sb[:, JCH + 1:JCH + 1 + B1], ctb)
                nc.vector.tensor_copy(
                    dbg_sb[:, JCH + 1 + B1:JCH + 1 + 2 * B1], ct_sb)
                nc.gpsimd.dma_start(out=dbg[:, :], in_=dbg_sb)
    return nc


_NC_CACHE = {}


def _get_nc(no_ar=False):
    if no_ar not in _NC_CACHE:
        nc = build(no_ar=no_ar)
        legalize_waits(nc)
        _NC_CACHE[no_ar] = nc
    return _NC_CACHE[no_ar]


def _make_in_maps(survtime, censor, hazard_pred):
    s = np.ascontiguousarray(np.asarray(survtime, np.float32).reshape(-1))
    cen = np.ascontiguousarray(np.asarray(censor, np.float32).reshape(-1))
    th = np.ascontiguousarray(np.asarray(hazard_pred, np.float32).reshape(-1))
    assert s.shape == (N,) and cen.shape == (N,) and th.shape == (N,)

    p = np.arange(P, dtype=np.float32)
    vp1 = p[:, None].astype(np.float32)      # [v_i <= p] grid
    ibc = ((p - B1) / np.float32(B1))[:, None].astype(np.float32)
    bg = np.where(p < B1, p, -5.0)[:, None].astype(np.float32)
    one = np.ones((P, 1), np.float32)
    scl = np.full((P, 1), 1.0 / N, np.float32)
    iota_row = np.concatenate([
        np.arange(B2, dtype=np.float32),
        np.arange(B1, dtype=np.float32)])[None, :]

    in_maps = []
    for r in range(NCORES):
        sl = slice(r * MY, (r + 1) * MY)
        s_cm = np.ascontiguousarray(s[sl].reshape(JCH, P).T)
        th_cm = np.ascontiguousarray(th[sl].reshape(JCH, P).T)
        cen_cm = np.ascontiguousarray(cen[sl].reshape(JCH, P).T)
        pack = np.concatenate([s_cm, th_cm, cen_cm, vp1, ibc, bg, one, scl],
                              axis=1)
        assert pack.shape == (P, PACKW)
        in_maps.append({
            "in_pack": np.ascontiguousarray(pack),
            "in_row": np.ascontiguousarray(s[sl][None, :]),
            "in_iota": np.ascontiguousarray(iota_row),
        })
    return in_maps


def run(survtime, censor, hazard_pred, **kw):
    in_maps = _make_in_maps(survtime, censor, hazard_pred)
    res = run_bass_kernel_spmd(_get_nc(), in_maps, list(range(NCORES)), **kw)
    total = np.float64(0.0)
    for r in range(NCORES):
        total += np.float64(np.asarray(res.results[r]["partial"]).reshape(-1)[0])
    return np.asarray(total, dtype=np.float32), res


def kernel(survtime, censor, hazard_pred):
    loss, _ = run(survtime, censor, hazard_pred)
    return loss

